# revision 36
# baseline (speedup 1.0000x reference)
"""Two-layer Keras-style LSTM (recurrent dropout) on 8 TRN2 NeuronCores.

Strategy: tensor-parallel over the 4U gate dimension (each core owns 128
u-dims of every gate), all state kept transposed [dim, batch].  Per scan
step each core computes its h-slice and broadcasts it (raw + pre-masked)
to all peers with remote_dma_broadcast into sender-indexed slot buffers.
The two layer scans are fused into one software pipeline; the L2 input
projection (h1 @ W2) is computed in-scan and accumulated directly into
the L2 gate PSUM group.  The L1 input projection (x @ W1) is a separate
token-parallel phase whose output streams through DRAM and is prefetched
per-step during the scan.

Self-contained: hardcodes B=64, T=256, E=512, U=1024, 8 cores.
"""
import numpy as np

import concourse.bacc as bacc
import concourse.mybir as mybir
from concourse.bass_utils import run_bass_kernel_spmd

import ml_dtypes

BF16 = ml_dtypes.bfloat16

B = 64
E = 512
U = 1024
NCORES = 8
SL = U // NCORES  # 128 u-dims per core per gate
F32 = mybir.dt.float32
BFD = mybir.dt.bfloat16
SIG = mybir.ActivationFunctionType.Sigmoid
TANH = mybir.ActivationFunctionType.Tanh
ADD = mybir.AluOpType.add
MUL = mybir.AluOpType.mult


def build(T=256):
    TOK = T * B              # tokens, t-major
    NN = min(512, TOK)       # phase-A matmul moving size
    TC = TOK // NN           # phase-A token chunks
    XW_DEPTH = 8             # xw prefetch ring depth (steps)
    SB1 = 5 * B              # send cols L1: raw + 4 masked (bf16)
    SB2 = 4 * B              # send cols L2: 4 masked

    nc = bacc.Bacc("TRN2", target_bir_lowering=False, debug=False)

    # ---------------- DRAM ----------------
    xTs_d = nc.dram_tensor("xTs", [E, TOK], BFD, kind="ExternalInput")
    w1_d = nc.dram_tensor("w1loc", [E, 4 * SL], BFD, kind="ExternalInput")
    u1_d = nc.dram_tensor("u1loc", [U, 4 * SL], BFD, kind="ExternalInput")
    w2_d = nc.dram_tensor("w2loc", [U, 4 * SL], BFD, kind="ExternalInput")
    u2_d = nc.dram_tensor("u2loc", [U, 4 * SL], BFD, kind="ExternalInput")
    m1_d = nc.dram_tensor("m1own", [SL, 4 * B], BFD, kind="ExternalInput")
    m2_d = nc.dram_tensor("m2own", [SL, 4 * B], BFD, kind="ExternalInput")
    b1_d = nc.dram_tensor("b1loc", [SL, 4], F32, kind="ExternalInput")
    b2_d = nc.dram_tensor("b2loc", [SL, 4], F32, kind="ExternalInput")

    xw1_d = nc.dram_tensor("xw1T", [4, 128, TOK], F32)  # internal scratch
    hs_d = nc.dram_tensor("out_hs", [T, SL, B], F32, kind="ExternalOutput")
    c2_d = nc.dram_tensor("c2out", [SL, B], F32, kind="ExternalOutput")

    # ---------------- SBUF ----------------
    A = nc.alloc_sbuf_tensor
    xa_s = [A(f"xa{i}", [128, 4 * NN], BFD) for i in range(2)]  # x chunk ring
    w1_s = [A(f"w1s{k}", [128, 4 * SL], BFD) for k in range(4)]
    u1_s = [A(f"u1s{r}", [128, 4 * SL], BFD) for r in range(8)]
    u2_s = [A(f"u2s{r}", [128, 4 * SL], BFD) for r in range(8)]
    w2_s = [A(f"w2s{r}", [128, 4 * SL], BFD) for r in range(8)]
    m1_s = A("m1s", [128, 4 * B], BFD)
    m2_s = A("m2s", [128, 4 * B], BFD)
    b1_s = A("b1s", [128, 4], F32)
    b2_s = A("b2s", [128, 4], F32)

    stageA = [A(f"stageA{i}", [128, NN], F32) for i in range(2)]
    xw_ring = A("xw_ring", [128, XW_DEPTH * 4 * B], F32)

    slots1 = [A(f"slots1_{p}", [128, NCORES * SB1], BFD) for p in range(3)]
    slots2 = [A(f"slots2_{p}", [128, NCORES * SB2], BFD) for p in range(3)]
    sb1 = [A(f"sb1_{p}", [128, SB1], BFD) for p in range(2)]
    sb2 = [A(f"sb2_{p}", [128, SB2], BFD) for p in range(2)]

    z1_s = [A(f"z1_{p}", [128, 4 * B], F32) for p in range(2)]
    g1_s = [A(f"g1_{p}", [128, 4 * B], F32) for p in range(2)]
    g2_s = [A(f"g2_{p}", [128, 4 * B], F32) for p in range(2)]
    c1_s = A("c1s", [128, B], F32)
    c2_s = A("c2s", [128, B], F32)
    tanh1_s = A("tanh1s", [128, B], F32)   # tanh(c1), ACT -> DVE
    tanh2_s = A("tanh2s", [128, B], F32)
    tmpa_s = A("tmpas", [128, B], F32)     # DVE-local temps
    tmpb_s = A("tmpbs", [128, B], F32)
    h1_s = A("h1s", [128, B], F32)         # h1 fp32 (DVE-local)
    h2f_s = [A(f"h2f_{p}", [128, B], F32) for p in range(2)]

    # ---------------- PSUM ----------------
    psA = [nc.alloc_psum_tensor(f"psA{i}", [128, NN], F32) for i in range(2)]
    ps1 = [nc.alloc_psum_tensor(f"ps1_{p}", [128, 4 * B], F32) for p in range(2)]
    ps2 = [nc.alloc_psum_tensor(f"ps2_{p}", [128, 4 * B], F32) for p in range(2)]

    # ---------------- semaphores ----------------
    S = nc.alloc_semaphore
    in_sem = S("in_sem")
    xin = [S(f"xin{s}") for s in range(2)]
    peA, cpA = S("peA"), S("cpA")
    stA = [S(f"stA{s}") for s in range(2)]
    rs1 = [S(f"rs1_{p}") for p in range(3)]
    rs2 = [S(f"rs2_{p}") for p in range(3)]
    ls1, ls2 = S("ls1"), S("ls2")
    prep = S("prep")
    h1m, h2m, h2f = S("h1m"), S("h2m"), S("h2f")
    xws = [S(f"xws{s}") for s in range(XW_DEPTH)]
    zs1 = S("zs1")
    as1, as2 = S("as1"), S("as2")
    cs1, cs2 = S("cs1"), S("cs2")
    ts1, ts2 = S("ts1"), S("ts2")
    pe1, pe2 = S("pe1"), S("pe2")
    odma = [S(f"odma{s}") for s in range(2)]

    N_IN = 4 + 8 + 8 + 8 + 1 + 1 + 1 + 1  # init DMA count = 32

    def wg(g, sem, v):
        if v > 0:
            g.wait_ge(sem, v)

    def xw_ready_idx(t):
        return ((t * B) // NN + 1) * 4  # stores covering step t (tc-outer, m-inner)

    with nc.Block() as blk:

        # ================= SYNC: all plain DMA =================
        @blk.sync
        def _(g):
            for k in range(4):
                g.dma_start(w1_s[k][:], w1_d[k * 128:(k + 1) * 128, :]).then_inc(in_sem, 16)
            for r in range(8):
                g.dma_start(u1_s[r][:], u1_d[r * 128:(r + 1) * 128, :]).then_inc(in_sem, 16)
            for r in range(8):
                g.dma_start(u2_s[r][:], u2_d[r * 128:(r + 1) * 128, :]).then_inc(in_sem, 16)
            for r in range(8):
                g.dma_start(w2_s[r][:], w2_d[r * 128:(r + 1) * 128, :]).then_inc(in_sem, 16)
            g.dma_start(m1_s[:], m1_d[:]).then_inc(in_sem, 16)
            g.dma_start(m2_s[:], m2_d[:]).then_inc(in_sem, 16)
            g.dma_start(b1_s[:], b1_d[:]).then_inc(in_sem, 16)
            g.dma_start(b2_s[:], b2_d[:]).then_inc(in_sem, 16)

            # phase A: interleave x chunk loads with xw stores
            for k in range(4):
                g.dma_start(xa_s[0][:, k * NN:(k + 1) * NN],
                            xTs_d[k * 128:(k + 1) * 128, 0:NN]).then_inc(xin[0], 16)
            if TC > 1:
                for k in range(4):
                    g.dma_start(xa_s[1][:, k * NN:(k + 1) * NN],
                                xTs_d[k * 128:(k + 1) * 128, NN:2 * NN]).then_inc(xin[1], 16)
            for idx in range(TC * 4):
                tc, m = divmod(idx, 4)
                g.wait_ge(cpA, idx + 1)
                g.dma_start(xw1_d[m, :, tc * NN:(tc + 1) * NN],
                            stageA[idx % 2][:]).then_inc(stA[idx % 2], 16)
                if m == 3 and tc + 2 < TC:
                    wg(g, peA, 4 * (tc + 1))  # PE done with chunk tc -> ring slot free
                    for k in range(4):
                        g.dma_start(
                            xa_s[tc % 2][:, k * NN:(k + 1) * NN],
                            xTs_d[k * 128:(k + 1) * 128, (tc + 2) * NN:(tc + 3) * NN],
                        ).then_inc(xin[tc % 2], 16)

            # scan: xw prefetch interleaved with output writes
            for i in range(T + 1):
                if i < T:
                    ridx = xw_ready_idx(i)
                    g.wait_ge(stA[0], 16 * (ridx // 2))
                    g.wait_ge(stA[1], 16 * (ridx // 2))
                    wg(g, zs1, i - XW_DEPTH + 1)
                    for m in range(4):
                        g.dma_start(
                            xw_ring[:, (i % XW_DEPTH) * 4 * B + m * B:
                                    (i % XW_DEPTH) * 4 * B + (m + 1) * B],
                            xw1_d[m, :, i * B:(i + 1) * B],
                        ).then_inc(xws[i % XW_DEPTH], 16)
                if i >= 1:
                    j = i - 1
                    g.wait_ge(h2f, j + 1)
                    g.dma_start(hs_d[j], h2f_s[j % 2][:]).then_inc(odma[j % 2], 16)
            g.wait_ge(cs2, T)
            g.dma_start(c2_d[:], c2_s[:]).then_inc(odma[0], 16)
            g.wait_ge(odma[0], 16 * ((T + 1) // 2 + 1))
            g.wait_ge(odma[1], 16 * (T // 2))

        # ================= POOL: remote broadcasts =================
        @blk.gpsimd
        def _(g):
            pid = g.partition_id()
            npr = 0
            g.wait_ge(in_sem, 16 * N_IN)
            for i in range(T):
                p3 = i % 3
                for c in range(NCORES):
                    with g.If_eq(pid, c):
                        g.remote_dma_broadcast(
                            out_ap=slots1[p3][:, c * SB1:(c + 1) * SB1],
                            in_ap=sb1[i % 2][:],
                            remote_sem=rs1[p3], local_sem=ls1,
                            rdests=[(0, j) for j in range(NCORES)],
                        ).then_inc(prep, 1)
                npr += 1
                g.wait_ge(prep, npr)
                g.wait_ge(h1m, i + 1)
                g.trigger_dma(1)
                for c in range(NCORES):
                    with g.If_eq(pid, c):
                        g.remote_dma_broadcast(
                            out_ap=slots2[p3][:, c * SB2:(c + 1) * SB2],
                            in_ap=sb2[(i - 1) % 2][:],
                            remote_sem=rs2[p3], local_sem=ls2,
                            rdests=[(0, j) for j in range(NCORES)],
                        ).then_inc(prep, 1)
                npr += 1
                g.wait_ge(prep, npr)
                g.wait_ge(h2m, i + 1)  # h2m has +1 baseline from init memset
                g.trigger_dma(1)

        # ================= PE: matmuls =================
        @blk.tensor
        def _(g):
            g.wait_ge(in_sem, 16 * N_IN)
            for idx in range(TC * 4):
                tc, m = divmod(idx, 4)
                wg(g, cpA, idx - 1)
                if m == 0:
                    g.wait_ge(xin[tc % 2], 64 * (tc // 2 + 1))
                for k in range(4):
                    mm = g.matmul(
                        psA[idx % 2][:],
                        lhsT=w1_s[k][:, m * SL:(m + 1) * SL],
                        rhs=xa_s[tc % 2][:, k * NN:(k + 1) * NN],
                        start=(k == 0), stop=(k == 3),
                    )
                mm.then_inc(peA, 1)
            for i in range(T + 1):
                p3 = (i - 1) % 3
                if i >= 1:
                    g.wait_ge(rs1[p3], 16 * ((i - 1) // 3 + 1))
                if 1 <= i <= T - 1:
                    wg(g, zs1, i - 1)
                    for r in range(8):
                        for q in range(4):
                            mm = g.matmul(
                                ps1[i % 2][:, q * B:(q + 1) * B],
                                lhsT=u1_s[r][:, q * SL:(q + 1) * SL],
                                rhs=slots1[p3][:, r * SB1 + (1 + q) * B:
                                               r * SB1 + (2 + q) * B],
                                start=(r == 0 and q == 0),
                                stop=(r == 7 and q == 3),
                            )
                    mm.then_inc(pe1, 1)
                if i >= 1:
                    wg(g, as2, i - 2)
                    for r in range(8):
                        for m in range(4):
                            g.matmul(
                                ps2[(i - 1) % 2][:, m * B:(m + 1) * B],
                                lhsT=w2_s[r][:, m * SL:(m + 1) * SL],
                                rhs=slots1[p3][:, r * SB1:r * SB1 + B],
                                start=(r == 0 and m == 0), stop=False,
                            )
                    g.wait_ge(rs2[p3], 16 * ((i - 1) // 3 + 1))
                    for r in range(8):
                        for q in range(4):
                            mm = g.matmul(
                                ps2[(i - 1) % 2][:, q * B:(q + 1) * B],
                                lhsT=u2_s[r][:, q * SL:(q + 1) * SL],
                                rhs=slots2[p3][:, r * SB2 + q * B:
                                               r * SB2 + (1 + q) * B],
                                start=False, stop=(r == 7 and q == 3),
                            )
                    mm.then_inc(pe2, 1)

        # ================= DVE =================
        @blk.vector
        def _(g):
            g.memset(sb2[0][:], 0.0)
            g.memset(sb2[1][:], 0.0).then_inc(h2m, 1)  # baseline for iter-0 dummy bcast2
            g.wait_ge(in_sem, 16 * N_IN)
            for idx in range(TC * 4):
                g.wait_ge(peA, idx + 1)
                wg(g, stA[idx % 2], 16 * (idx // 2))
                g.tensor_copy(stageA[idx % 2][:], psA[idx % 2][:]).then_inc(cpA, 1)
            for i in range(T + 1):
                if i <= T - 1:
                    wg(g, as1, i - 1)          # z1 parity reuse
                    g.wait_ge(xws[i % XW_DEPTH], 64 * (i // XW_DEPTH + 1))
                    if i == 0:
                        g.tensor_copy(z1_s[0][:], xw_ring[:, 0:4 * B]).then_inc(zs1, 1)
                    else:
                        g.wait_ge(pe1, i)
                        g.tensor_tensor(
                            z1_s[i % 2][:], ps1[i % 2][:],
                            xw_ring[:, (i % XW_DEPTH) * 4 * B:(i % XW_DEPTH + 1) * 4 * B],
                            op=ADD,
                        ).then_inc(zs1, 1)
                    g.wait_ge(as1, i + 1)
                    gi = g1_s[i % 2][:, 0:B]
                    gf = g1_s[i % 2][:, B:2 * B]
                    gc = g1_s[i % 2][:, 2 * B:3 * B]
                    go = g1_s[i % 2][:, 3 * B:4 * B]
                    if i == 0:
                        g.tensor_mul(c1_s[:], gi, gc).then_inc(cs1, 1)
                    else:
                        g.tensor_mul(tmpa_s[:], gf, c1_s[:])
                        g.tensor_mul(tmpb_s[:], gi, gc)
                        g.drain()
                        g.tensor_add(c1_s[:], tmpa_s[:], tmpb_s[:]).then_inc(cs1, 1)
                    g.wait_ge(ts1, i + 1)
                    if i >= 2:
                        g.wait_ge(ls1, 16 * i)  # sendbuf1 reuse: all prior sends drained
                    g.tensor_mul(h1_s[:], go, tanh1_s[:])
                    g.drain()
                    g.tensor_copy(sb1[i % 2][:, 0:B], h1_s[:])
                    for q in range(4):
                        tt = g.tensor_tensor(
                            sb1[i % 2][:, (1 + q) * B:(2 + q) * B],
                            h1_s[:], m1_s[:, q * B:(q + 1) * B], op=MUL,
                        )
                    tt.then_inc(h1m, 1)
                if i >= 1:
                    j = i - 1
                    g.wait_ge(as2, j + 1)
                    gi = g2_s[j % 2][:, 0:B]
                    gf = g2_s[j % 2][:, B:2 * B]
                    gc = g2_s[j % 2][:, 2 * B:3 * B]
                    go = g2_s[j % 2][:, 3 * B:4 * B]
                    if j == 0:
                        g.tensor_mul(c2_s[:], gi, gc).then_inc(cs2, 1)
                    else:
                        g.tensor_mul(tmpa_s[:], gf, c2_s[:])
                        g.tensor_mul(tmpb_s[:], gi, gc)
                        g.drain()
                        g.tensor_add(c2_s[:], tmpa_s[:], tmpb_s[:]).then_inc(cs2, 1)
                    g.wait_ge(ts2, j + 1)
                    wg(g, odma[j % 2], 16 * (j // 2))  # h2f parity reuse
                    g.tensor_mul(h2f_s[j % 2][:], go, tanh2_s[:]).then_inc(h2f, 1)
                    g.drain()
                    if i >= 2:
                        g.wait_ge(ls2, 16 * i)  # sendbuf2 reuse: all prior sends drained
                    for q in range(4):
                        tt = g.tensor_tensor(
                            sb2[j % 2][:, q * B:(q + 1) * B],
                            h2f_s[j % 2][:], m2_s[:, q * B:(q + 1) * B], op=MUL,
                        )
                    tt.then_inc(h2m, 1)

        # ================= ACT =================
        @blk.scalar
        def _(g):
            g.wait_ge(in_sem, 16 * N_IN)
            for i in range(T + 1):
                if i <= T - 1:
                    g.wait_ge(zs1, i + 1)
                    wg(g, cs1, i - 1)          # gates parity reuse
                    for q, fn in ((0, SIG), (1, SIG), (2, TANH), (3, SIG)):
                        aa = g.activation(
                            g1_s[i % 2][:, q * B:(q + 1) * B],
                            z1_s[i % 2][:, q * B:(q + 1) * B],
                            fn, bias=b1_s[:, q:q + 1],
                        )
                    aa.then_inc(as1, 1)
                    g.wait_ge(cs1, i + 1)
                    wg(g, h1m, i)              # tanh1 read by prev h-mul done
                    g.activation(tanh1_s[:], c1_s[:], TANH).then_inc(ts1, 1)
                if i >= 1:
                    j = i - 1
                    g.wait_ge(pe2, i)
                    wg(g, cs2, j - 1)
                    for q, fn in ((0, SIG), (1, SIG), (2, TANH), (3, SIG)):
                        aa = g.activation(
                            g2_s[j % 2][:, q * B:(q + 1) * B],
                            ps2[j % 2][:, q * B:(q + 1) * B],
                            fn, bias=b2_s[:, q:q + 1],
                        )
                    aa.then_inc(as2, 1)
                    g.wait_ge(cs2, j + 1)
                    wg(g, h2m, i)              # tanh2 read done (+1 baseline)
                    g.activation(tanh2_s[:], c2_s[:], TANH).then_inc(ts2, 1)

    nc.compile()
    return nc


# ====================== host side ======================

_CACHE = {}


def _get_nc(T):
    if T not in _CACHE:
        _CACHE[T] = build(T)
    return _CACHE[T]


def make_in_maps(x, W1, U1, b1, W2, U2, b2, m1, m2, T):
    xt = np.ascontiguousarray(np.transpose(x[:, :T, :], (1, 0, 2))).reshape(T * B, E)
    xT_host = np.ascontiguousarray(xt.T).astype(BF16)
    in_maps = []
    for c in range(NCORES):
        cols = np.concatenate(
            [np.arange(g * U + c * SL, g * U + (c + 1) * SL) for g in range(4)]
        )
        m1own = np.stack([m1[g, :, c * SL:(c + 1) * SL].T for g in range(4)], 1)
        m2own = np.stack([m2[g, :, c * SL:(c + 1) * SL].T for g in range(4)], 1)
        in_maps.append({
            "xTs": xT_host,
            "w1loc": np.ascontiguousarray(W1[:, cols]).astype(BF16),
            "u1loc": np.ascontiguousarray(U1[:, cols]).astype(BF16),
            "w2loc": np.ascontiguousarray(W2[:, cols]).astype(BF16),
            "u2loc": np.ascontiguousarray(U2[:, cols]).astype(BF16),
            "m1own": np.ascontiguousarray(m1own.reshape(SL, 4 * B)).astype(BF16),
            "m2own": np.ascontiguousarray(m2own.reshape(SL, 4 * B)).astype(BF16),
            "b1loc": np.ascontiguousarray(b1.reshape(4, U)[:, c * SL:(c + 1) * SL].T).astype(np.float32),
            "b2loc": np.ascontiguousarray(b2.reshape(4, U)[:, c * SL:(c + 1) * SL].T).astype(np.float32),
        })
    return in_maps


def assemble(results, T):
    hs = np.concatenate([r["out_hs"] for r in results], axis=1)  # [T, U, B]
    out2 = np.ascontiguousarray(np.transpose(hs, (2, 0, 1)))     # [B, T, U]
    h2 = np.ascontiguousarray(out2[:, T - 1, :])
    c2 = np.concatenate([r["c2out"] for r in results], axis=0).T  # [B, U]
    return out2, h2, np.ascontiguousarray(c2)


def kernel(x, W1, U1, b1, W2, U2, b2, m1, m2):
    args = [np.asarray(a, dtype=np.float32)
            for a in (x, W1, U1, b1, W2, U2, b2, m1, m2)]
    T = args[0].shape[1]
    nc = _get_nc(T)
    in_maps = make_in_maps(*args, T=T)
    last_err = None
    for _ in range(3):  # retry: a crashed prior run can leave a core wedged
        try:
            res = run_bass_kernel_spmd(nc, in_maps, core_ids=list(range(NCORES)))
            return assemble(res.results, T)
        except Exception as e:  # noqa: BLE001
            last_err = e
    raise last_err
